# revision 1
# baseline (speedup 1.0000x reference)
"""Trainium2 Bass kernel for nn_Mlp_StaticRoutedLoRAExpert.

Computation (per token chunk with static expert e):
    h = gelu(x @ w1.T + bias1 + SCALE * (x @ a1[e].T) @ b1[e].T)
    y = h @ w2.T + bias2 + SCALE * (h @ a2[e].T) @ b2[e].T

Sharding: data-parallel over batch, 4 batches per core on 8 cores, no
collectives.  Each core computes in feature-major layout (X^T, H^T, Y^T)
so biases live on partitions and the token dim is the matmul moving dim.

Two phases per core (W1^T and W2^T don't fit SBUF together at fp32):
  phase 1: fc1+gelu for all tokens -> H^T scratch in device DRAM
  phase 2: fc2 for all tokens -> Y^T

All matmuls run as float32r (full fp32 storage; relaxed fp32 PE mode,
1 cycle/row at N>=256 - measured ~1.4e-4 rel err vs fp64 reference).
"""

import numpy as np

SCALE = 128.0 / 64.0
B, S, IN, HID, OUT, E, R = 32, 1280, 768, 3072, 768, 2, 64
NCORES = 8
BPC = B // NCORES          # batches per core
TPC = BPC * S              # tokens per core
P = 128
KI = IN // P               # 6  input k-chunks
KH = HID // P              # 24 hidden chunks
KO = OUT // P              # 6  output chunks
MAX_T = 512                # fp32 moving-operand limit

_nc_cache: dict = {}


def _plan_tiles(chunk_sizes, expert_ids):
    """Per-core token tiles: (col_offset, n_tokens, expert)."""
    tiles = []
    for b in range(BPC):
        base = b * S
        start = 0
        for sz, e in zip(chunk_sizes, expert_ids):
            off = 0
            while off < sz:
                t = min(MAX_T, sz - off)
                tiles.append((base + start + off, t, int(e)))
                off += t
            start += sz
    return tuple(tiles)


def _build(tiles, debug_ht=False, timing_internal_io=False, timing_small=False):
    import concourse.bacc as bacc
    import concourse.mybir as mybir
    import concourse.tile as tile

    dt = mybir.dt
    f32 = dt.float32
    f32r = dt.float32r
    AF = mybir.ActivationFunctionType

    nc = bacc.Bacc("TRN2", target_bir_lowering=False, num_devices=NCORES)

    io_kind = "Internal" if timing_internal_io else "ExternalInput"
    out_kind = "Internal" if timing_internal_io else "ExternalOutput"
    tpc = 512 if timing_small else TPC
    xt_d = nc.dram_tensor("xt", [IN, tpc], f32, kind=io_kind)
    w1t_d = nc.dram_tensor("w1t", [IN, HID], f32, kind="ExternalInput")
    b1v_d = nc.dram_tensor("bias1", [HID], f32, kind="ExternalInput")
    a1t_d = nc.dram_tensor("a1t", [E, IN, R], f32, kind="ExternalInput")
    b1t_d = nc.dram_tensor("b1t", [E, R, HID], f32, kind="ExternalInput")
    w2t_d = nc.dram_tensor("w2t", [HID, OUT], f32, kind="ExternalInput")
    b2v_d = nc.dram_tensor("bias2", [OUT], f32, kind="ExternalInput")
    a2t_d = nc.dram_tensor("a2t", [E, HID, R], f32, kind="ExternalInput")
    b2t_d = nc.dram_tensor("b2t", [E, R, OUT], f32, kind="ExternalInput")
    yt_d = nc.dram_tensor("yt", [OUT, tpc], f32, kind=out_kind)
    probe_d = None
    if timing_internal_io:
        probe_d = nc.dram_tensor("probe", [P, KO], f32, kind="ExternalOutput")
    ht_d = nc.dram_tensor("htscr", [HID, tpc], f32,
                          kind="ExternalOutput" if debug_ht else "Internal")

    def rd(ap):
        return ap.bitcast(f32r)

    with tile.TileContext(nc) as tc:
        with tc.tile_pool(name="bias", bufs=1) as bias_pool:
            bias1_s = bias_pool.tile([P, KH], f32)
            nc.sync.dma_start(bias1_s[:], b1v_d.ap().rearrange("(c p) -> p c", p=P))
            bias2_s = bias_pool.tile([P, KO], f32)
            nc.sync.dma_start(bias2_s[:], b2v_d.ap().rearrange("(c p) -> p c", p=P))

            # w2t preloaded during phase 1 (fits alongside phase-1 working set)
            _w2_ctx = tc.tile_pool(name="w2", bufs=1)
            w2_pool = _w2_ctx.__enter__()
            w2t_s = w2_pool.tile([P, KH, OUT], f32r)
            nc.sync.dma_start(
                w2t_s[:], rd(w2t_d.ap().rearrange("(k p) o -> p k o", p=P))
            )

            # ---------------- phase 1: fc1 + gelu ----------------
            with (
                tc.tile_pool(name="w1", bufs=1) as w1_pool,
                tc.tile_pool(name="lora1", bufs=1) as lora1_pool,
                tc.tile_pool(name="xp", bufs=10) as xpool,
                tc.tile_pool(name="hp", bufs=5) as hpool,
                tc.tile_pool(name="u1p", bufs=2) as u1pool,
                tc.tile_pool(name="ps1", bufs=6, space="PSUM") as ps1,
                tc.tile_pool(name="psu1", bufs=2, space="PSUM") as psu1,
            ):
                w1t_s = w1_pool.tile([P, KI, HID], f32r)
                nc.sync.dma_start(
                    w1t_s[:], rd(w1t_d.ap().rearrange("(k p) h -> p k h", p=P))
                )
                a1t_s = lora1_pool.tile([P, E, KI, R], f32r)
                nc.sync.dma_start(
                    a1t_s[:], rd(a1t_d.ap().rearrange("e (k p) r -> p e k r", p=P))
                )
                b1t_s = lora1_pool.tile([R, E, HID], f32r)
                nc.sync.dma_start(b1t_s[:], rd(b1t_d.ap().rearrange("e r h -> r e h")))

                for (col0, T, e) in tiles:
                    col = (col0 % 512 if col0 % 512 + T <= 512 else 0) if timing_small else col0
                    xc = []
                    for k in range(KI):
                        xck = xpool.tile([P, T], f32r, name=f"xc{k}", tag="xc")
                        nc.sync.dma_start(
                            xck[:], rd(xt_d[k * P:(k + 1) * P, col:col + T])
                        )
                        xc.append(xck)
                    u1_ps = psu1.tile([R, T], f32, name="u1ps", tag="u1ps")
                    for k in range(KI):
                        nc.tensor.matmul(
                            u1_ps[:], a1t_s[:, e, k, :], xc[k][:],
                            start=(k == 0), stop=(k == KI - 1),
                        )
                    u1_s = u1pool.tile([R, T], f32r, name="u1s", tag="u1s")
                    nc.vector.tensor_copy(u1_s[:], u1_ps[:])
                    for m in range(KH):
                        h_ps = ps1.tile([P, T], f32, name="hps", tag="hps")
                        for k in range(KI):
                            nc.tensor.matmul(
                                h_ps[:],
                                w1t_s[:, k, m * P:(m + 1) * P],
                                xc[k][:],
                                start=(k == 0), stop=False,
                            )
                        nc.tensor.matmul(
                            h_ps[:],
                            b1t_s[:, e, m * P:(m + 1) * P],
                            u1_s[:],
                            start=False, stop=True,
                        )
                        hc = hpool.tile([P, T], f32r, name="hc", tag="hc")
                        nc.scalar.activation(
                            hc[:], h_ps[:], AF.Gelu, bias=bias1_s[:, m:m + 1]
                        )
                        nc.sync.dma_start(
                            rd(ht_d[m * P:(m + 1) * P, col:col + T]), hc[:]
                        )


            # ---------------- phase 2: fc2 ----------------
            with (
                tc.tile_pool(name="lora2", bufs=1) as lora2_pool,
                tc.tile_pool(name="hp2", bufs=KH + 12) as hpool2,
                tc.tile_pool(name="yp", bufs=8) as ypool,
                tc.tile_pool(name="u2p", bufs=3) as u2pool,
                tc.tile_pool(name="ps2", bufs=6, space="PSUM") as ps2,
                tc.tile_pool(name="psu2", bufs=2, space="PSUM") as psu2,
            ):
                a2t_s = lora2_pool.tile([P, E, KH, R], f32r)
                nc.sync.dma_start(
                    a2t_s[:], rd(a2t_d.ap().rearrange("e (k p) r -> p e k r", p=P))
                )
                b2t_s = lora2_pool.tile([R, E, OUT], f32r)
                nc.sync.dma_start(b2t_s[:], rd(b2t_d.ap().rearrange("e r o -> r e o")))

                for (col0, T, e) in tiles:
                    col = (col0 % 512 if col0 % 512 + T <= 512 else 0) if timing_small else col0
                    hcs = []
                    for m in range(KH):
                        hcm = hpool2.tile([P, T], f32r, name=f"h2_{m}", tag="h2")
                        nc.sync.dma_start(
                            hcm[:], rd(ht_d[m * P:(m + 1) * P, col:col + T])
                        )
                        hcs.append(hcm)
                    u2_ps = psu2.tile([R, T], f32, name="u2ps", tag="u2ps")
                    for m in range(KH):
                        nc.tensor.matmul(
                            u2_ps[:], a2t_s[:, e, m, :], hcs[m][:],
                            start=(m == 0), stop=(m == KH - 1),
                        )
                    u2_s = u2pool.tile([R, T], f32r, name="u2s", tag="u2s")
                    nc.vector.tensor_copy(u2_s[:], u2_ps[:])
                    for o in range(KO):
                        y_ps = ps2.tile([P, T], f32, name="yps", tag="yps")
                        for m in range(KH):
                            nc.tensor.matmul(
                                y_ps[:],
                                w2t_s[:, m, o * P:(o + 1) * P],
                                hcs[m][:],
                                start=(m == 0), stop=False,
                            )
                        nc.tensor.matmul(
                            y_ps[:],
                            b2t_s[:, e, o * P:(o + 1) * P],
                            u2_s[:],
                            start=False, stop=True,
                        )
                        yc = ypool.tile([P, T], f32, name="yc", tag="yc")
                        nc.scalar.activation(
                            yc[:], y_ps[:], AF.Identity, bias=bias2_s[:, o:o + 1]
                        )
                        nc.sync.dma_start(
                            yt_d[o * P:(o + 1) * P, col:col + T], yc[:]
                        )

            _w2_ctx.__exit__(None, None, None)

        if probe_d is not None:
            nc.sync.dma_start(probe_d.ap(), yt_d[0:P, 0:KO])
    nc.compile()
    return nc


def _get_nc(tiles):
    nc = _nc_cache.get(tiles)
    if nc is None:
        nc = _nc_cache[tiles] = _build(tiles)
    return nc


def _run(inputs, trace=False):
    from concourse.bass_utils import run_bass_kernel_spmd

    x = np.asarray(inputs["x"], dtype=np.float32)
    w1 = np.asarray(inputs["w1"], dtype=np.float32)
    bias1 = np.asarray(inputs["bias1"], dtype=np.float32)
    a1 = np.asarray(inputs["a1"], dtype=np.float32)
    b1 = np.asarray(inputs["b1"], dtype=np.float32)
    w2 = np.asarray(inputs["w2"], dtype=np.float32)
    bias2 = np.asarray(inputs["bias2"], dtype=np.float32)
    a2 = np.asarray(inputs["a2"], dtype=np.float32)
    b2 = np.asarray(inputs["b2"], dtype=np.float32)
    chunk_sizes = tuple(int(v) for v in np.asarray(inputs["chunk_sizes"]))
    eids = tuple(int(v) for v in np.asarray(inputs["expert_indices"]))
    assert sum(chunk_sizes) == S

    tiles = _plan_tiles(chunk_sizes, eids)
    nc = _get_nc(tiles)

    xT = np.ascontiguousarray(x.reshape(B * S, IN).T)
    shared = {
        "w1t": np.ascontiguousarray(w1.T),
        "bias1": bias1,
        "a1t": np.ascontiguousarray(a1.transpose(0, 2, 1)),
        "b1t": np.ascontiguousarray((SCALE * b1).transpose(0, 2, 1)),
        "w2t": np.ascontiguousarray(w2.T),
        "bias2": bias2,
        "a2t": np.ascontiguousarray(a2.transpose(0, 2, 1)),
        "b2t": np.ascontiguousarray((SCALE * b2).transpose(0, 2, 1)),
    }
    in_maps = []
    for c in range(NCORES):
        m = dict(shared)
        m["xt"] = np.ascontiguousarray(xT[:, c * TPC:(c + 1) * TPC])
        in_maps.append(m)

    res = run_bass_kernel_spmd(
        nc, in_maps, core_ids=list(range(NCORES)), trace=trace
    )
    yT = np.concatenate([r["yt"] for r in res.results], axis=1)
    y = np.ascontiguousarray(yT.T).reshape(B, S, OUT)
    return y, res


def kernel(**inputs) -> np.ndarray:
    y, _ = _run(inputs, trace=False)
    return y



# revision 2
# speedup vs baseline: 1.2313x; 1.2313x over previous
"""Trainium2 Bass kernel for nn_Mlp_StaticRoutedLoRAExpert.

Computation (per token chunk with static expert e):
    h = gelu(x @ w1.T + bias1 + SCALE * (x @ a1[e].T) @ b1[e].T)
    y = h @ w2.T + bias2 + SCALE * (h @ a2[e].T) @ b2[e].T

Since experts are static per chunk, the LoRA factors fold into the dense
weights on the host (exact math, done in float64):
    W1eff[e] = w1 + SCALE * b1[e] @ a1[e]        # [HID, IN]
    W2eff[e] = w2 + SCALE * b2[e] @ a2[e]        # [OUT, HID]
so the device kernel is a plain two-layer MLP with a per-chunk weight
select.  All device tensors are bf16 (1 cycle/row on the PE array, same
as fp32r, half the SBUF/DMA) except PSUM accumulation (f32) and biases.

Sharding: data-parallel over batch, 4 batches per core on 8 cores, no
collectives.  Feature-major layout (X^T, H^T, Y^T): features on
partitions, tokens as the matmul moving dim.  Per 512-token tile:
  fc1 (6x24 matmuls) -> gelu+bias1 on Act -> H^T in SBUF (never DRAM)
  fc2 (24x6 matmuls into 6 PSUM banks) -> bias2+cast drain on Vector
x / y are pre-blocked on host so each tile is ONE contiguous DMA.
"""

import numpy as np
import ml_dtypes

SCALE = 128.0 / 64.0
B, S, IN, HID, OUT, E, R = 32, 1280, 768, 3072, 768, 2, 64
NCORES = 8
BPC = B // NCORES          # batches per core
TPC = BPC * S              # tokens per core
P = 128
KI = IN // P               # 6  input k-chunks
KH = HID // P              # 24 hidden chunks
KO = OUT // P              # 6  output chunks
MAX_T = 512                # one PSUM bank of f32 per matmul output

BF16 = ml_dtypes.bfloat16

_nc_cache: dict = {}


def _plan_tiles(chunk_sizes, expert_ids):
    """Per-core token tiles: (col_offset, n_tokens, expert)."""
    tiles = []
    for b in range(BPC):
        base = b * S
        start = 0
        for sz, e in zip(chunk_sizes, expert_ids):
            off = 0
            while off < sz:
                t = min(MAX_T, sz - off)
                tiles.append((base + start + off, t, int(e)))
                off += t
            start += sz
    return tuple(tiles)


def _build(tiles):
    import concourse.bacc as bacc
    import concourse.mybir as mybir
    import concourse.tile as tile

    dt = mybir.dt
    f32 = dt.float32
    bf16 = dt.bfloat16
    AF = mybir.ActivationFunctionType

    nc = bacc.Bacc("TRN2", target_bir_lowering=False, num_devices=NCORES)

    xb_d = nc.dram_tensor("xb", [P, KI * TPC], bf16, kind="ExternalInput")
    w1t_d = nc.dram_tensor("w1t", [P, E, KI, HID], bf16, kind="ExternalInput")
    w2t_d = nc.dram_tensor("w2t", [P, E, KH, OUT], bf16, kind="ExternalInput")
    b1v_d = nc.dram_tensor("bias1", [P, KH], f32, kind="ExternalInput")
    b2v_d = nc.dram_tensor("bias2", [P, KO], f32, kind="ExternalInput")
    yb_d = nc.dram_tensor("yb", [P, KO * TPC], bf16, kind="ExternalOutput")

    with tile.TileContext(nc) as tc:
        with (
            tc.tile_pool(name="const", bufs=1) as cpool,
            tc.tile_pool(name="w", bufs=1) as wpool,
            tc.tile_pool(name="xp", bufs=2) as xpool,
            tc.tile_pool(name="hp", bufs=KH + 2) as hpool,
            tc.tile_pool(name="yp", bufs=2) as ypool,
            tc.tile_pool(name="psh", bufs=2, space="PSUM") as psh,
            tc.tile_pool(name="psy", bufs=KO, space="PSUM") as psy,
        ):
            bias1_s = cpool.tile([P, KH], f32)
            nc.sync.dma_start(bias1_s[:], b1v_d.ap())
            bias2_s = cpool.tile([P, KO], f32)
            nc.sync.dma_start(bias2_s[:], b2v_d.ap())
            w1_s = wpool.tile([P, E, KI, HID], bf16)
            nc.sync.dma_start(w1_s[:], w1t_d.ap())
            w2_s = wpool.tile([P, E, KH, OUT], bf16)
            nc.sync.dma_start(w2_s[:], w2t_d.ap())

            for (col0, T, e) in tiles:
                x_s = xpool.tile([P, KI * T], bf16, name="x", tag="x")
                nc.sync.dma_start(
                    x_s[:], xb_d[:, KI * col0:KI * (col0 + T)]
                )
                # fc1 + gelu: H^T chunks stay in SBUF
                hcs = []
                for m in range(KH):
                    h_ps = psh.tile([P, T], f32, name="hps", tag="hps")
                    for k in range(KI):
                        nc.tensor.matmul(
                            h_ps[:],
                            w1_s[:, e, k, m * P:(m + 1) * P],
                            x_s[:, k * T:(k + 1) * T],
                            start=(k == 0), stop=(k == KI - 1),
                        )
                    hc = hpool.tile([P, T], bf16, name="hc", tag="hc")
                    nc.scalar.activation(
                        hc[:], h_ps[:], AF.Gelu, bias=bias1_s[:, m:m + 1]
                    )
                    hcs.append(hc)
                # fc2: per output chunk, contiguous accumulation in one bank
                y_s = ypool.tile([P, KO * T], bf16, name="y", tag="y")
                for o in range(KO):
                    y_ps = psy.tile([P, T], f32, name="yps", tag="yps")
                    for m in range(KH):
                        nc.tensor.matmul(
                            y_ps[:],
                            w2_s[:, e, m, o * P:(o + 1) * P],
                            hcs[m][:],
                            start=(m == 0), stop=(m == KH - 1),
                        )
                    nc.vector.tensor_scalar_add(
                        y_s[:, o * T:(o + 1) * T], y_ps[:], bias2_s[:, o:o + 1]
                    )
                nc.sync.dma_start(
                    yb_d[:, KO * col0:KO * (col0 + T)], y_s[:]
                )
    nc.compile()
    return nc


def _get_nc(tiles):
    nc = _nc_cache.get(tiles)
    if nc is None:
        nc = _nc_cache[tiles] = _build(tiles)
    return nc


def _prep_shared(inputs):
    """Merge LoRA into dense weights (f64), transpose, tile, cast bf16."""
    w1 = np.asarray(inputs["w1"], dtype=np.float64)
    b1 = np.asarray(inputs["b1"], dtype=np.float64)
    a1 = np.asarray(inputs["a1"], dtype=np.float64)
    w2 = np.asarray(inputs["w2"], dtype=np.float64)
    b2 = np.asarray(inputs["b2"], dtype=np.float64)
    a2 = np.asarray(inputs["a2"], dtype=np.float64)

    w1t = np.empty((P, E, KI, HID), dtype=BF16)
    w2t = np.empty((P, E, KH, OUT), dtype=BF16)
    for e in range(E):
        w1e = (w1 + SCALE * (b1[e] @ a1[e])).T        # [IN, HID]
        w1t[:, e] = w1e.reshape(KI, P, HID).transpose(1, 0, 2).astype(BF16)
        w2e = (w2 + SCALE * (b2[e] @ a2[e])).T        # [HID, OUT]
        w2t[:, e] = w2e.reshape(KH, P, OUT).transpose(1, 0, 2).astype(BF16)

    bias1 = np.asarray(inputs["bias1"], dtype=np.float32)
    bias2 = np.asarray(inputs["bias2"], dtype=np.float32)
    return {
        "w1t": w1t,
        "w2t": w2t,
        "bias1": np.ascontiguousarray(bias1.reshape(KH, P).T),
        "bias2": np.ascontiguousarray(bias2.reshape(KO, P).T),
    }


def _run(inputs, trace=False):
    from concourse.bass_utils import run_bass_kernel_spmd

    x = np.asarray(inputs["x"], dtype=np.float32)
    chunk_sizes = tuple(int(v) for v in np.asarray(inputs["chunk_sizes"]))
    eids = tuple(int(v) for v in np.asarray(inputs["expert_indices"]))
    assert sum(chunk_sizes) == S

    tiles = _plan_tiles(chunk_sizes, eids)
    nc = _get_nc(tiles)
    shared = _prep_shared(inputs)

    xbf = x.astype(BF16).reshape(NCORES, TPC, IN)
    in_maps = []
    for c in range(NCORES):
        xb = np.empty((P, KI * TPC), dtype=BF16)
        for (col0, T, _e) in tiles:
            blk = xbf[c, col0:col0 + T].reshape(T, KI, P).transpose(2, 1, 0)
            xb[:, KI * col0:KI * (col0 + T)] = blk.reshape(P, KI * T)
        m = dict(shared)
        m["xb"] = xb
        in_maps.append(m)

    res = run_bass_kernel_spmd(
        nc, in_maps, core_ids=list(range(NCORES)), trace=trace
    )

    y = np.empty((NCORES, TPC, OUT), dtype=np.float32)
    for c in range(NCORES):
        yb = res.results[c]["yb"]
        for (col0, T, _e) in tiles:
            t0 = KO * col0
            blk = yb[:, t0:t0 + KO * T].reshape(P, KO, T)
            y[c, col0:col0 + T] = (
                blk.transpose(2, 1, 0).reshape(T, OUT).astype(np.float32)
            )
    return y.reshape(B, S, OUT), res


def kernel(**inputs) -> np.ndarray:
    y, _ = _run(inputs, trace=False)
    return y


# revision 6
# speedup vs baseline: 1.3123x; 1.0658x over previous
"""Trainium2 Bass kernel for nn_Mlp_StaticRoutedLoRAExpert.

Computation (per token chunk with static expert e):
    h = gelu(x @ w1.T + bias1 + SCALE * (x @ a1[e].T) @ b1[e].T)
    y = h @ w2.T + bias2 + SCALE * (h @ a2[e].T) @ b2[e].T

Since experts are static per chunk, the LoRA factors fold into the dense
weights on the host (exact math, done in float64):
    W1eff[e] = w1 + SCALE * b1[e] @ a1[e]        # [HID, IN]
    W2eff[e] = w2 + SCALE * b2[e] @ a2[e]        # [OUT, HID]
so the device kernel is a plain two-layer MLP with a per-chunk weight
select.  All device tensors are bf16 (1 cycle/row on the PE array, same
as fp32r, half the SBUF/DMA) except PSUM accumulation (f32) and biases.

Sharding: data-parallel over batch, 4 batches per core on 8 cores, no
collectives.  Feature-major layout (X^T, H^T, Y^T): features on
partitions, tokens as the matmul moving dim.  Per 512-token tile:
  fc1 (6x24 matmuls) -> gelu+bias1 on Act -> H^T in SBUF (never DRAM)
  fc2 (24x6 matmuls into 6 PSUM banks) -> bias2+cast drain on Vector
x / y are pre-blocked on host so each tile is ONE contiguous DMA.
"""

import numpy as np
import ml_dtypes

SCALE = 128.0 / 64.0
B, S, IN, HID, OUT, E, R = 32, 1280, 768, 3072, 768, 2, 64
NCORES = 8
BPC = B // NCORES          # batches per core
TPC = BPC * S              # tokens per core
P = 128
KI = IN // P               # 6  input k-chunks
KH = HID // P              # 24 hidden chunks
KO = OUT // P              # 6  output chunks
MAX_T = 512                # one PSUM bank of f32 per matmul output

BF16 = ml_dtypes.bfloat16

_nc_cache: dict = {}


def _plan_tiles(chunk_sizes, expert_ids):
    """Per-core token tiles: (col_offset, n_tokens, expert)."""
    tiles = []
    for b in range(BPC):
        base = b * S
        start = 0
        for sz, e in zip(chunk_sizes, expert_ids):
            off = 0
            while off < sz:
                t = min(MAX_T, sz - off)
                tiles.append((base + start + off, t, int(e)))
                off += t
            start += sz
    return tuple(tiles)


def _build(tiles):
    import concourse.bacc as bacc
    import concourse.mybir as mybir
    import concourse.tile as tile

    dt = mybir.dt
    f32 = dt.float32
    bf16 = dt.bfloat16
    AF = mybir.ActivationFunctionType

    nc = bacc.Bacc("TRN2", target_bir_lowering=False, num_devices=NCORES)

    xb_d = nc.dram_tensor("xb", [P, KI * TPC], bf16, kind="ExternalInput")
    w1t_d = nc.dram_tensor("w1t", [E, KI, P, HID], bf16, kind="ExternalInput")
    w2t_d = nc.dram_tensor("w2t", [E, P, KH, OUT], bf16, kind="ExternalInput")
    b1v_d = nc.dram_tensor("bias1", [P, KH], f32, kind="ExternalInput")
    b2v_d = nc.dram_tensor("bias2", [P, KO], f32, kind="ExternalInput")
    yb_d = nc.dram_tensor("yb", [P, KO * TPC], bf16, kind="ExternalOutput")

    # load weights in first-use order so tile 0 stalls only on its own
    # expert's first chunks, not on the full 18 MB weight set
    expert_order = list(dict.fromkeys([e for (_, _, e) in tiles]))
    expert_order += [e for e in range(E) if e not in expert_order]

    with tile.TileContext(nc) as tc:
        with (
            tc.tile_pool(name="const", bufs=1) as cpool,
            tc.tile_pool(name="w", bufs=1) as wpool,
            tc.tile_pool(name="xp", bufs=2) as xpool,
            tc.tile_pool(name="hp", bufs=KH + 2) as hpool,
            tc.tile_pool(name="yp", bufs=2) as ypool,
            tc.tile_pool(name="psh", bufs=2, space="PSUM") as psh,
            tc.tile_pool(name="psy", bufs=KO, space="PSUM") as psy,
        ):
            bias1_s = cpool.tile([P, KH], f32)
            nc.sync.dma_start(bias1_s[:], b1v_d.ap())
            bias2_s = cpool.tile([P, KO], f32)
            nc.sync.dma_start(bias2_s[:], b2v_d.ap())
            w1_s = [[None] * KI for _ in range(E)]
            w2_s = [None] * E
            for e in expert_order:
                for k in range(KI):
                    w1_s[e][k] = wpool.tile([P, HID], bf16, name=f"w1_{e}_{k}")
                    nc.sync.dma_start(w1_s[e][k][:], w1t_d[e, k])
                w2_s[e] = wpool.tile([P, KH, OUT], bf16, name=f"w2_{e}")
                nc.sync.dma_start(w2_s[e][:], w2t_d[e])

            for (col0, T, e) in tiles:
                x_s = xpool.tile([P, KI * T], bf16, name="x", tag="x")
                # x/y ride the Activation HWDGE ring so they never queue
                # behind the weight stream on the SP ring
                nc.scalar.dma_start(
                    x_s[:], xb_d[:, KI * col0:KI * (col0 + T)]
                )
                # fc1 + gelu: H^T chunks stay in SBUF
                hcs = []
                for m in range(KH):
                    h_ps = psh.tile([P, T], f32, name="hps", tag="hps")
                    for k in range(KI):
                        nc.tensor.matmul(
                            h_ps[:],
                            w1_s[e][k][:, m * P:(m + 1) * P],
                            x_s[:, k * T:(k + 1) * T],
                            start=(k == 0), stop=(k == KI - 1),
                        )
                    hc = hpool.tile([P, T], bf16, name="hc", tag="hc")
                    nc.scalar.activation(
                        hc[:], h_ps[:], AF.Gelu, bias=bias1_s[:, m:m + 1]
                    )
                    hcs.append(hc)
                # fc2: per output chunk, contiguous accumulation in one bank
                y_s = ypool.tile([P, KO * T], bf16, name="y", tag="y")
                for o in range(KO):
                    y_ps = psy.tile([P, T], f32, name="yps", tag="yps")
                    for m in range(KH):
                        nc.tensor.matmul(
                            y_ps[:],
                            w2_s[e][:, m, o * P:(o + 1) * P],
                            hcs[m][:],
                            start=(m == 0), stop=(m == KH - 1),
                        )
                    nc.vector.tensor_scalar_add(
                        y_s[:, o * T:(o + 1) * T], y_ps[:], bias2_s[:, o:o + 1]
                    )
                nc.scalar.dma_start(
                    yb_d[:, KO * col0:KO * (col0 + T)], y_s[:]
                )
    nc.compile()
    return nc


def _get_nc(tiles):
    nc = _nc_cache.get(tiles)
    if nc is None:
        nc = _nc_cache[tiles] = _build(tiles)
    return nc


def _prep_shared(inputs):
    """Merge LoRA into dense weights (f64), transpose, tile, cast bf16."""
    w1 = np.asarray(inputs["w1"], dtype=np.float64)
    b1 = np.asarray(inputs["b1"], dtype=np.float64)
    a1 = np.asarray(inputs["a1"], dtype=np.float64)
    w2 = np.asarray(inputs["w2"], dtype=np.float64)
    b2 = np.asarray(inputs["b2"], dtype=np.float64)
    a2 = np.asarray(inputs["a2"], dtype=np.float64)

    w1t = np.empty((E, KI, P, HID), dtype=BF16)
    w2t = np.empty((E, P, KH, OUT), dtype=BF16)
    for e in range(E):
        w1e = (w1 + SCALE * (b1[e] @ a1[e])).T        # [IN, HID]
        w1t[e] = w1e.reshape(KI, P, HID).astype(BF16)
        w2e = (w2 + SCALE * (b2[e] @ a2[e])).T        # [HID, OUT]
        w2t[e] = w2e.reshape(KH, P, OUT).transpose(1, 0, 2).astype(BF16)

    bias1 = np.asarray(inputs["bias1"], dtype=np.float32)
    bias2 = np.asarray(inputs["bias2"], dtype=np.float32)
    return {
        "w1t": w1t,
        "w2t": w2t,
        "bias1": np.ascontiguousarray(bias1.reshape(KH, P).T),
        "bias2": np.ascontiguousarray(bias2.reshape(KO, P).T),
    }


def _make_in_maps(inputs, tiles):
    x = np.asarray(inputs["x"], dtype=np.float32)
    shared = _prep_shared(inputs)
    xbf = x.astype(BF16).reshape(NCORES, TPC, IN)
    in_maps = []
    for c in range(NCORES):
        xb = np.empty((P, KI * TPC), dtype=BF16)
        for (col0, T, _e) in tiles:
            blk = xbf[c, col0:col0 + T].reshape(T, KI, P).transpose(2, 1, 0)
            xb[:, KI * col0:KI * (col0 + T)] = blk.reshape(P, KI * T)
        m = dict(shared)
        m["xb"] = xb
        in_maps.append(m)
    return in_maps


def _assemble_y(results, tiles):
    y = np.empty((NCORES, TPC, OUT), dtype=np.float32)
    for c in range(NCORES):
        yb = results[c]["yb"]
        for (col0, T, _e) in tiles:
            t0 = KO * col0
            blk = yb[:, t0:t0 + KO * T].reshape(P, KO, T)
            y[c, col0:col0 + T] = (
                blk.transpose(2, 1, 0).reshape(T, OUT).astype(np.float32)
            )
    return y.reshape(B, S, OUT)


def _run(inputs, trace=False):
    from concourse.bass_utils import run_bass_kernel_spmd

    chunk_sizes = tuple(int(v) for v in np.asarray(inputs["chunk_sizes"]))
    eids = tuple(int(v) for v in np.asarray(inputs["expert_indices"]))
    assert sum(chunk_sizes) == S

    tiles = _plan_tiles(chunk_sizes, eids)
    nc = _get_nc(tiles)
    in_maps = _make_in_maps(inputs, tiles)

    res = run_bass_kernel_spmd(
        nc, in_maps, core_ids=list(range(NCORES)), trace=trace
    )
    return _assemble_y(res.results, tiles), res


def kernel(**inputs) -> np.ndarray:
    y, _ = _run(inputs, trace=False)
    return y
